# revision 12
# baseline (speedup 1.0000x reference)
"""CosLoss (ArcFace-style margin loss) Trainium2 kernel, 8-way class-sharded.

Math (reference):
    xn   = x / ||x||_row                       [B, D]
    wf   = xn @ W.T                            [B, C]
    corr = wf[i, labels[i]]                    [B]
    num  = S*(corr - M)
    excl = sum_j exp(S*wf[i,j]) - exp(S*corr)
    L    = num - log(exp(num) + excl);  out = -mean(L)

Device does the O(B*C*D) matmul and the O(B*C) exp row-sums; host does the
O(B*D) glue exactly in fp64 (row norms -> rs = S/||x||, the correct-class
dot, and the final scalar combine).

Per core (classes sharded 8 x 4000, padded to 4096):
  - z = x @ W_shard.T via fp8(e4m3) DoubleRow matmuls: the full D=256
    contraction in one PE pass (2 fp8 weights/cell), ~1.7x bf16 throughput.
    x is scaled by SX, W by SW on host; rs/(SX*SW) undoes it in the exp.
  - Row sums of exp(rs*z) are split across two engines working in parallel:
      ACT: exact Exp activation with fused accumulate (scale=rs).
      DVE: custom 8-stage op computing (1 + y/32)^32 ~= exp(y) with a fused
           accumulator (one 1x pass per tile, no separate reduce). The
           systematic bias of the pow-32 approximation is removed on host
           with a single scale factor calibrated on a 256-column sample.
  - Outputs are per-(batch-tile, half) partial sums; host combines.
"""

import math
from contextlib import ExitStack
from operator import add as _op_add

import ml_dtypes
import numpy as np

import concourse.bass as bass
import concourse.mybir as mybir
import concourse.tile as tile
from concourse import bacc
from concourse.bass_utils import run_bass_kernel_spmd

S = 30.0
MARGIN = 0.4
N_CORES = 8
B, D, C = 8192, 256, 32000
CSH = C // N_CORES          # 4000 real classes per core
CPAD = 4096                 # padded shard width
NPAD = CPAD - CSH           # 96 zero-padded classes (always in half 1)
P = 128
NBT = B // P                # 64 batch tiles
HALF = CPAD // 2            # 2048 classes per PSUM tile
SX, SW = 4.0, 32.0          # fp8 pre-scales for x and W
KEXP = 32.0                 # (1 + y/32)^32 fastexp on DVE

FP32 = mybir.dt.float32
BF16 = mybir.dt.bfloat16
FP8 = mybir.dt.float8e4
NP_FP8 = ml_dtypes.float8_e4m3

NCHUNK = 4                  # 1024-wide PSUM chunks per bt
CW = CPAD // NCHUNK         # 1024 columns per chunk


# Consumer of each (bt, chunk): ACT takes chunks {0,1}, DVE {2,3}, except
# every 8th bt where ACT only takes {0} (120:136 split matches the engines'
# measured per-chunk rates). Chunk 3 (which holds all zero-padding) is
# always DVE.
def _is_act(bt, ch):
    if ch >= 2:
        return False
    if ch == 1 and bt % 8 == 7:
        return False
    return True


# ---------------------------------------------------------------------------
# Custom DVE op: out = (in0*s0 + s1)^32, accum_out = sum(out) along free dim.
# Body depth 7 (mul, add, 5x square) + accumulator stage 8.
# Registered into concourse.dve_ops at import time (name-keyed registry).
# ---------------------------------------------------------------------------
_POW32_NAME = "POW32_EXP_REDUCE_ANT"


def _pow32_ref(in0, in1, c0, c1, c2):
    b = (in0.astype(np.float32) * c0 + c1).astype(np.float32)
    for _ in range(5):
        b = (b * b).astype(np.float32)
    return b, b.reshape(b.shape[0], -1).sum(axis=-1, keepdims=True)


def _register_pow32_op():
    import concourse.dve_ops as dve_ops
    from concourse.dve_spec import C0, C1, Spec, Src0, Zero, _has_src1, lower, sq
    from concourse.dve_uop import DveOpSpec

    if any(op.name == _POW32_NAME for op in dve_ops.OPS):
        return next(op for op in dve_ops.OPS if op.name == _POW32_NAME)

    body = Src0 * C0 + C1
    for _ in range(5):
        body = sq(body)
    spec = Spec(body=body, accum=_op_add, accum_init=Zero, reference=_pow32_ref)

    row = dve_ops._CUSTOM_DVE_ROW_BASE + len(dve_ops.OPS)
    assert row < 0x20
    shas = {}
    for ver in ("v3", "v4"):
        try:
            s = DveOpSpec(
                name=_POW32_NAME, opcode=row, uops=lower(spec, ver=ver),
                rd1_en=_has_src1(spec),
            )
            shas[ver] = s.sha(ver)
        except Exception:
            pass
    assert "v3" in shas, "pow32 spec failed to lower for TRN2 (v3)"
    op = dve_ops.DveOp(_POW32_NAME, spec, subdim=False, uops_sha=shas)
    dve_ops.OPS.append(op)
    dve_ops.CUSTOM_DVE_SPECS[_POW32_NAME] = spec
    dve_ops._SUB_OPCODE_FOR_NAME[_POW32_NAME] = row
    return op


_POW32_OP = _register_pow32_op()


def _emit(tc, ins, outs):
    nc = tc.nc
    xT8, wT8, rs_sc, rs_k = ins["xT8"], ins["wT8"], ins["rs_sc"], ins["rs_k"]
    parts_a, parts_d = outs["parts_a"], outs["parts_d"]

    with ExitStack() as ctx:
        singles = ctx.enter_context(tc.tile_pool(name="singles", bufs=1))
        scr = ctx.enter_context(tc.tile_pool(name="scr", bufs=2))
        psum = ctx.enter_context(tc.tile_pool(name="psum", bufs=4, space="PSUM"))

        rs_sc_sb = singles.tile([P, NBT], FP32)
        rs_k_sb = singles.tile([P, NBT], FP32)
        wT_sb = singles.tile([P, 2, CPAD], FP8)
        xT_sb = singles.tile([P, 2, B], FP8)
        pa_sb = singles.tile([P, NCHUNK * NBT], FP32)
        pd_sb = singles.tile([P, NCHUNK * NBT], FP32)
        nc.gpsimd.memset(pa_sb, 0.0)
        nc.gpsimd.memset(pd_sb, 0.0)

        # Prologue DMAs spread across engine DGE queues (each queue moves
        # only ~26 GB/s); ordered so bt 0's deps land first.
        nc.sync.dma_start(out=rs_sc_sb, in_=rs_sc.rearrange("(t p) -> p t", p=P))
        nc.sync.dma_start(out=rs_k_sb, in_=rs_k.rearrange("(t p) -> p t", p=P))
        wt_engines = [nc.sync, nc.scalar, nc.gpsimd, nc.scalar]
        for j, eng in enumerate(wt_engines):
            eng.dma_start(
                out=wT_sb[:, :, j * CW : (j + 1) * CW],
                in_=wT8[:, :, j * CW : (j + 1) * CW],
            )
        nxc = 8
        for c in range(nxc):
            lo, hi = c * (B // nxc), (c + 1) * (B // nxc)
            eng = nc.gpsimd if c % 2 == 0 else nc.sync
            eng.dma_start(out=xT_sb[:, :, lo:hi], in_=xT8[:, :, lo:hi])

        for bt in range(NBT):
            lhs = xT_sb[:, :, bt * P : (bt + 1) * P]
            for ch in range(NCHUNK):
                pt = psum.tile([P, CW], FP32, tag="pt")
                for j in range(2):
                    c0 = ch * CW + j * 512
                    nc.tensor.matmul(
                        pt[:, j * 512 : (j + 1) * 512],
                        lhsT=lhs,
                        rhs=wT_sb[:, :, c0 : c0 + 512],
                        start=True,
                        stop=True,
                        perf_mode=mybir.MatmulPerfMode.DoubleRow,
                    )
                slot = NCHUNK * bt + ch
                if _is_act(bt, ch):
                    et = scr.tile([P, CW], BF16, tag="et")
                    nc.scalar.activation(
                        out=et, in_=pt, func=mybir.ActivationFunctionType.Exp,
                        scale=rs_sc_sb[:, bt : bt + 1],
                        accum_out=pa_sb[:, slot : slot + 1],
                    )
                else:
                    ft = scr.tile([P, CW], BF16, tag="ft")
                    nc.vector._custom_dve(
                        _POW32_OP,
                        out=ft,
                        in0=pt,
                        s0=rs_k_sb[:, bt : bt + 1],
                        s1=1.0,
                        accum_out=pd_sb[:, slot : slot + 1],
                    )

        nc.sync.dma_start(out=parts_a.rearrange("(t p) -> p t", p=P), in_=pa_sb)
        nc.sync.dma_start(out=parts_d.rearrange("(t p) -> p t", p=P), in_=pd_sb)


def _build():
    nc = bacc.Bacc("TRN2", target_bir_lowering=False, debug=False)
    ins = {
        "xT8": nc.dram_tensor("xT8", [P, 2, B], FP8, kind="ExternalInput").ap(),
        "wT8": nc.dram_tensor("wT8", [P, 2, CPAD], FP8, kind="ExternalInput").ap(),
        "rs_sc": nc.dram_tensor("rs_sc", [B], FP32, kind="ExternalInput").ap(),
        "rs_k": nc.dram_tensor("rs_k", [B], FP32, kind="ExternalInput").ap(),
    }
    outs = {
        "parts_a": nc.dram_tensor(
            "parts_a", [NCHUNK * NBT * P], FP32, kind="ExternalOutput"
        ).ap(),
        "parts_d": nc.dram_tensor(
            "parts_d", [NCHUNK * NBT * P], FP32, kind="ExternalOutput"
        ).ap(),
    }
    with tile.TileContext(nc) as tc:
        _emit(tc, ins, outs)
    nc.compile()
    return nc


_NC_CACHE = {}


def _get_nc():
    if "nc" not in _NC_CACHE:
        _NC_CACHE["nc"] = _build()
    return _NC_CACHE["nc"]


def _install_trace_hook():
    """Make `antenv.axon_hooks` importable so run_bass_kernel_spmd(trace=True)
    can capture NTFF profiles under axon. Returns False if unavailable."""
    try:
        from antenv.axon_hooks import get_axon_ntff_profile_hook  # noqa: F401

        return True
    except ImportError:
        pass
    try:
        import sys
        import types

        from trn_agent_boot.trn_boot import _ntff_profile_via_ctypes

        hook = _ntff_profile_via_ctypes("/opt/axon/libaxon_pjrt.so")
        if hook is None:
            return False
        mod = types.ModuleType("antenv.axon_hooks")
        mod._hook = hook
        mod.get_axon_ntff_profile_hook = lambda: mod._hook
        mod.set_axon_ntff_profile_hook = lambda h: setattr(mod, "_hook", h)
        sys.modules["antenv.axon_hooks"] = mod
        import antenv

        antenv.axon_hooks = mod
        return True
    except Exception:
        return False


def _pack_T(a8):
    """[N, 256] fp8 row-major -> [128, 2, N] DoubleRow operand layout:
    out[p, kk, n] = a8[n, kk*128 + p]."""
    n = a8.shape[0]
    return np.ascontiguousarray(a8.reshape(n, 2, P).transpose(2, 1, 0))


def _fastexp_pow32(y):
    b = (y.astype(np.float32) * np.float32(1.0 / KEXP) + np.float32(1.0)).astype(
        np.float32
    )
    for _ in range(5):
        b = (b * b).astype(np.float32)
    return b


def kernel(x, labels, W, trace=False):
    x = np.ascontiguousarray(np.asarray(x, dtype=np.float32))
    W = np.ascontiguousarray(np.asarray(W, dtype=np.float32))
    labels_i = np.asarray(labels).astype(np.int64)

    # Host glue (exact, O(B*D)): row norms, rs, correct-class logit.
    ssq = np.einsum("bd,bd->b", x.astype(np.float64), x.astype(np.float64))
    rs = S / np.sqrt(ssq)                                     # [B] fp64
    dotg = np.einsum(
        "bd,bd->b", x.astype(np.float64), W[labels_i].astype(np.float64)
    )
    scorr = rs * dotg
    num = scorr - S * MARGIN

    # Device operands.
    x8 = (x * SX).astype(NP_FP8)
    xT8 = _pack_T(x8)
    rs_dev = (rs / (SX * SW)).astype(np.float32)
    rs_sc = rs_dev
    rs_k = (rs_dev / KEXP).astype(np.float32)

    in_maps = []
    w8_f32 = None
    for k in range(N_CORES):
        w8k = (W[k * CSH : (k + 1) * CSH] * SW).astype(NP_FP8)
        if k == 0:
            w8_f32 = w8k.astype(np.float32)  # for calibration sampling
        w8p = np.zeros((CPAD, D), dtype=NP_FP8)
        w8p[:CSH] = w8k
        in_maps.append(
            {"xT8": xT8, "wT8": _pack_T(w8p), "rs_sc": rs_sc, "rs_k": rs_k}
        )

    nc = _get_nc()
    if trace and not _install_trace_hook():
        trace = False
    res = run_bass_kernel_spmd(nc, in_maps, core_ids=list(range(N_CORES)), trace=trace)
    if trace and res.exec_time_ns is not None:
        print(f"HW exec time: {res.exec_time_ns} ns")

    # Calibrate the pow-32 fastexp bias on a 256-column sample of core 0's
    # shard (same fp8 values the device saw).
    x8_f32 = x8.astype(np.float32)
    cols = np.arange(0, CSH, CSH // 256)[:256]
    z_s = x8_f32 @ w8_f32[cols].T                              # [B, 256] fp32
    y_s = rs_dev[:, None] * z_s
    ratio = float(_fastexp_pow32(y_s).astype(np.float64).sum()) / float(
        np.exp(y_s.astype(np.float64)).sum()
    )

    # Combine per-(bt, chunk) partials. Zero-padded classes (NPAD columns,
    # always in chunk 3 -> DVE) contribute exactly 1.0 under both exp and
    # pow32 (since (1+0)^32 == 1).
    slots = np.arange(NCHUNK * NBT)
    bt_of_slot = slots // NCHUNK
    ch_of_slot = slots % NCHUNK
    act_mask = np.array(
        [_is_act(b, c) for b, c in zip(bt_of_slot, ch_of_slot)], dtype=bool
    )

    sum_a = np.zeros((P, NCHUNK * NBT))
    sum_d = np.zeros((P, NCHUNK * NBT))
    for r in res.results:
        sum_a += r["parts_a"].reshape(NCHUNK * NBT, P).T.astype(np.float64)
        sum_d += r["parts_d"].reshape(NCHUNK * NBT, P).T.astype(np.float64)

    pad_d = np.where((ch_of_slot == NCHUNK - 1), NPAD * N_CORES, 0)
    sum_d = sum_d - pad_d[None, :]
    sum_d = np.where(act_mask[None, :], 0.0, sum_d) / ratio
    sum_a = np.where(act_mask[None, :], sum_a, 0.0)

    per_bt = (sum_a + sum_d).reshape(P, NBT, NCHUNK).sum(2)    # [P, NBT]
    rowsum = per_bt.T.reshape(B)                               # row i = bt*128+p

    excl = rowsum - np.exp(scorr)
    L = num - np.log(np.exp(num) + excl)
    return np.float32(-np.mean(L))


# revision 17
# speedup vs baseline: 1.6486x; 1.6486x over previous
"""CosLoss (ArcFace-style margin loss) Trainium2 kernel, 8-way class-sharded.

Math (reference):
    xn   = x / ||x||_row                       [B, D]
    wf   = xn @ W.T                            [B, C]
    corr = wf[i, labels[i]]                    [B]
    num  = S*(corr - M)
    excl = sum_j exp(S*wf[i,j]) - exp(S*corr)
    L    = num - log(exp(num) + excl);  out = -mean(L)

Device does the O(B*C*D) matmul and the O(B*C) exp row-sums; host does the
O(B*D) glue exactly in fp64 (row norms -> rs = S/||x||, the correct-class
dot, and the final scalar combine).

Per core (classes sharded 8 x 4000, padded to 4096):
  - z = x @ W_shard.T via fp8(e4m3) DoubleRow matmuls: the full D=256
    contraction in one PE pass (2 fp8 weights/cell), ~1.7x bf16 throughput.
    x is scaled by SX, W by SW on host; rs/(SX*SW) undoes it in the exp.
  - Row sums of exp(rs*z) are split across two engines working in parallel:
      ACT: exact Exp activation with fused accumulate (scale=rs).
      DVE: custom 8-stage op computing (1 + y/32)^32 ~= exp(y) with a fused
           accumulator (one 1x pass per tile, no separate reduce). The
           systematic bias of the pow-32 approximation is removed on host
           with a single scale factor calibrated on a 256-column sample.
  - Outputs are per-(batch-tile, half) partial sums; host combines.
"""

import math
from contextlib import ExitStack
from operator import add as _op_add

import ml_dtypes
import numpy as np

import concourse.bass as bass
import concourse.mybir as mybir
import concourse.tile as tile
from concourse import bacc
from concourse.bass_utils import run_bass_kernel_spmd

S = 30.0
MARGIN = 0.4
N_CORES = 8
B, D, C = 8192, 256, 32000
CSH = C // N_CORES          # 4000 real classes per core
CPAD = 4096                 # padded shard width
NPAD = CPAD - CSH           # 96 zero-padded classes (always in half 1)
P = 128
NBT = B // P                # 64 batch tiles
HALF = CPAD // 2            # 2048 classes per PSUM tile
SX, SW = 4.0, 32.0          # fp8 pre-scales for x and W
KEXP = 32.0                 # (1 + y/32)^32 fastexp on DVE

FP32 = mybir.dt.float32
BF16 = mybir.dt.bfloat16
FP8 = mybir.dt.float8e4
NP_FP8 = ml_dtypes.float8_e4m3

NCHUNK = 4                  # 1024-wide PSUM chunks per bt
CW = CPAD // NCHUNK         # 1024 columns per chunk


# Consumer of each (bt, chunk): ACT takes chunks {0,1}, DVE {2,3}, except
# every 8th bt where ACT only takes {0} (120:136 split matches the engines'
# measured per-chunk rates). Chunk 3 (which holds all zero-padding) is
# always DVE.
def _is_act(bt, ch):
    if ch >= 2:
        return False
    if ch == 1 and bt % 8 == 7:
        return False
    return True


# ---------------------------------------------------------------------------
# Custom DVE op: out = (in0*s0 + s1)^32, accum_out = sum(out) along free dim.
# Body depth 7 (mul, add, 5x square) + accumulator stage 8.
# Registered into concourse.dve_ops at import time (name-keyed registry).
# ---------------------------------------------------------------------------
_POW32_NAME = "POW32_EXP_REDUCE_ANT"


def _pow32_ref(in0, in1, c0, c1, c2):
    b = (in0.astype(np.float32) * c0 + c1).astype(np.float32)
    for _ in range(5):
        b = (b * b).astype(np.float32)
    return b, b.reshape(b.shape[0], -1).sum(axis=-1, keepdims=True)


def _register_pow32_op():
    import concourse.dve_ops as dve_ops
    from concourse.dve_spec import C0, C1, Spec, Src0, Zero, _has_src1, lower, sq
    from concourse.dve_uop import DveOpSpec

    if any(op.name == _POW32_NAME for op in dve_ops.OPS):
        return next(op for op in dve_ops.OPS if op.name == _POW32_NAME)

    body = Src0 * C0 + C1
    for _ in range(5):
        body = sq(body)
    spec = Spec(body=body, accum=_op_add, accum_init=Zero, reference=_pow32_ref)

    row = dve_ops._CUSTOM_DVE_ROW_BASE + len(dve_ops.OPS)
    assert row < 0x20
    shas = {}
    for ver in ("v3", "v4"):
        try:
            s = DveOpSpec(
                name=_POW32_NAME, opcode=row, uops=lower(spec, ver=ver),
                rd1_en=_has_src1(spec),
            )
            shas[ver] = s.sha(ver)
        except Exception:
            pass
    assert "v3" in shas, "pow32 spec failed to lower for TRN2 (v3)"
    op = dve_ops.DveOp(_POW32_NAME, spec, subdim=False, uops_sha=shas)
    dve_ops.OPS.append(op)
    dve_ops.CUSTOM_DVE_SPECS[_POW32_NAME] = spec
    dve_ops._SUB_OPCODE_FOR_NAME[_POW32_NAME] = row
    return op


_POW32_OP = _register_pow32_op()


def _emit(tc, ins, outs):
    nc = tc.nc
    xT8, wT8, rs_sc, rs_k = ins["xT8"], ins["wT8"], ins["rs_sc"], ins["rs_k"]
    parts_a, parts_d = outs["parts_a"], outs["parts_d"]

    with ExitStack() as ctx:
        singles = ctx.enter_context(tc.tile_pool(name="singles", bufs=1))
        scr = ctx.enter_context(tc.tile_pool(name="scr", bufs=2))
        psum = ctx.enter_context(tc.tile_pool(name="psum", bufs=4, space="PSUM"))

        rs_sc_sb = singles.tile([P, NBT], FP32)
        rs_k_sb = singles.tile([P, NBT], FP32)
        wT_sb = singles.tile([P, 2, CPAD], FP8)
        xT_sb = singles.tile([P, 2, B], FP8)
        pa_sb = singles.tile([P, NCHUNK * NBT], FP32)
        pd_sb = singles.tile([P, NCHUNK * NBT], FP32)
        nc.gpsimd.memset(pa_sb, 0.0)
        nc.gpsimd.memset(pd_sb, 0.0)

        # Prologue DMAs spread across engine DGE queues (each queue moves
        # only ~26 GB/s); ordered so bt 0's deps land first. All host<->dev
        # layouts are SBUF-native [P, ...] so every DMA is contiguous per
        # partition (element-scatter patterns cost ~60us in descriptors).
        nc.sync.dma_start(out=rs_sc_sb, in_=rs_sc)
        nc.sync.dma_start(out=rs_k_sb, in_=rs_k)
        wt_engines = [nc.sync, nc.scalar, nc.gpsimd, nc.scalar]
        for j, eng in enumerate(wt_engines):
            eng.dma_start(
                out=wT_sb[:, :, j * CW : (j + 1) * CW],
                in_=wT8[:, :, j * CW : (j + 1) * CW],
            )
        nxc = 8
        for c in range(nxc):
            lo, hi = c * (B // nxc), (c + 1) * (B // nxc)
            eng = nc.gpsimd if c % 2 == 0 else nc.sync
            eng.dma_start(out=xT_sb[:, :, lo:hi], in_=xT8[:, :, lo:hi])

        for bt in range(NBT):
            lhs = xT_sb[:, :, bt * P : (bt + 1) * P]
            for ch in range(NCHUNK):
                pt = psum.tile([P, CW], FP32, tag="pt")
                for j in range(2):
                    c0 = ch * CW + j * 512
                    nc.tensor.matmul(
                        pt[:, j * 512 : (j + 1) * 512],
                        lhsT=lhs,
                        rhs=wT_sb[:, :, c0 : c0 + 512],
                        start=True,
                        stop=True,
                        perf_mode=mybir.MatmulPerfMode.DoubleRow,
                    )
                slot = NCHUNK * bt + ch
                if _is_act(bt, ch):
                    et = scr.tile([P, CW], BF16, tag="et")
                    nc.scalar.activation(
                        out=et, in_=pt, func=mybir.ActivationFunctionType.Exp,
                        scale=rs_sc_sb[:, bt : bt + 1],
                        accum_out=pa_sb[:, slot : slot + 1],
                    )
                else:
                    ft = scr.tile([P, CW], BF16, tag="ft")
                    nc.vector._custom_dve(
                        _POW32_OP,
                        out=ft,
                        in0=pt,
                        s0=rs_k_sb[:, bt : bt + 1],
                        s1=1.0,
                        accum_out=pd_sb[:, slot : slot + 1],
                    )

        nc.sync.dma_start(out=parts_a, in_=pa_sb)
        nc.scalar.dma_start(out=parts_d, in_=pd_sb)


def _build():
    nc = bacc.Bacc("TRN2", target_bir_lowering=False, debug=False)
    ins = {
        "xT8": nc.dram_tensor("xT8", [P, 2, B], FP8, kind="ExternalInput").ap(),
        "wT8": nc.dram_tensor("wT8", [P, 2, CPAD], FP8, kind="ExternalInput").ap(),
        "rs_sc": nc.dram_tensor("rs_sc", [P, NBT], FP32, kind="ExternalInput").ap(),
        "rs_k": nc.dram_tensor("rs_k", [P, NBT], FP32, kind="ExternalInput").ap(),
    }
    outs = {
        "parts_a": nc.dram_tensor(
            "parts_a", [P, NCHUNK * NBT], FP32, kind="ExternalOutput"
        ).ap(),
        "parts_d": nc.dram_tensor(
            "parts_d", [P, NCHUNK * NBT], FP32, kind="ExternalOutput"
        ).ap(),
    }
    with tile.TileContext(nc) as tc:
        _emit(tc, ins, outs)
    nc.compile()
    return nc


_NC_CACHE = {}


def _get_nc():
    if "nc" not in _NC_CACHE:
        _NC_CACHE["nc"] = _build()
    return _NC_CACHE["nc"]


def _install_trace_hook():
    """Make `antenv.axon_hooks` importable so run_bass_kernel_spmd(trace=True)
    can capture NTFF profiles under axon. Returns False if unavailable."""
    try:
        from antenv.axon_hooks import get_axon_ntff_profile_hook  # noqa: F401

        return True
    except ImportError:
        pass
    try:
        import sys
        import types

        from trn_agent_boot.trn_boot import _ntff_profile_via_ctypes

        hook = _ntff_profile_via_ctypes("/opt/axon/libaxon_pjrt.so")
        if hook is None:
            return False
        mod = types.ModuleType("antenv.axon_hooks")
        mod._hook = hook
        mod.get_axon_ntff_profile_hook = lambda: mod._hook
        mod.set_axon_ntff_profile_hook = lambda h: setattr(mod, "_hook", h)
        sys.modules["antenv.axon_hooks"] = mod
        import antenv

        antenv.axon_hooks = mod
        return True
    except Exception:
        return False


def _pack_T(a8):
    """[N, 256] fp8 row-major -> [128, 2, N] DoubleRow operand layout:
    out[p, kk, n] = a8[n, kk*128 + p]."""
    n = a8.shape[0]
    return np.ascontiguousarray(a8.reshape(n, 2, P).transpose(2, 1, 0))


def _fastexp_pow32(y):
    b = (y.astype(np.float32) * np.float32(1.0 / KEXP) + np.float32(1.0)).astype(
        np.float32
    )
    for _ in range(5):
        b = (b * b).astype(np.float32)
    return b


def kernel(x, labels, W, trace=False):
    x = np.ascontiguousarray(np.asarray(x, dtype=np.float32))
    W = np.ascontiguousarray(np.asarray(W, dtype=np.float32))
    labels_i = np.asarray(labels).astype(np.int64)

    # Host glue (exact, O(B*D)): row norms, rs, correct-class logit.
    ssq = np.einsum("bd,bd->b", x.astype(np.float64), x.astype(np.float64))
    rs = S / np.sqrt(ssq)                                     # [B] fp64
    dotg = np.einsum(
        "bd,bd->b", x.astype(np.float64), W[labels_i].astype(np.float64)
    )
    scorr = rs * dotg
    num = scorr - S * MARGIN

    # Device operands.
    x8 = (x * SX).astype(NP_FP8)
    xT8 = _pack_T(x8)
    rs_dev = (rs / (SX * SW)).astype(np.float32)
    # [P, NBT] SBUF-native layout: [p, bt] = rs[bt*128 + p]
    rs_sc = np.ascontiguousarray(rs_dev.reshape(NBT, P).T)
    rs_k = np.ascontiguousarray((rs_dev / KEXP).astype(np.float32).reshape(NBT, P).T)

    in_maps = []
    w8_f32 = None
    for k in range(N_CORES):
        w8k = (W[k * CSH : (k + 1) * CSH] * SW).astype(NP_FP8)
        if k == 0:
            w8_f32 = w8k.astype(np.float32)  # for calibration sampling
        w8p = np.zeros((CPAD, D), dtype=NP_FP8)
        w8p[:CSH] = w8k
        in_maps.append(
            {"xT8": xT8, "wT8": _pack_T(w8p), "rs_sc": rs_sc, "rs_k": rs_k}
        )

    nc = _get_nc()
    if trace and not _install_trace_hook():
        trace = False
    res = run_bass_kernel_spmd(nc, in_maps, core_ids=list(range(N_CORES)), trace=trace)
    if trace and res.exec_time_ns is not None:
        print(f"HW exec time: {res.exec_time_ns} ns")

    # Calibrate the pow-32 fastexp bias on a 256-column sample of core 0's
    # shard (same fp8 values the device saw).
    x8_f32 = x8.astype(np.float32)
    cols = np.arange(0, CSH, CSH // 256)[:256]
    z_s = x8_f32 @ w8_f32[cols].T                              # [B, 256] fp32
    y_s = rs_dev[:, None] * z_s
    ratio = float(_fastexp_pow32(y_s).astype(np.float64).sum()) / float(
        np.exp(y_s.astype(np.float64)).sum()
    )

    # Combine per-(bt, chunk) partials. Zero-padded classes (NPAD columns,
    # always in chunk 3 -> DVE) contribute exactly 1.0 under both exp and
    # pow32 (since (1+0)^32 == 1).
    slots = np.arange(NCHUNK * NBT)
    bt_of_slot = slots // NCHUNK
    ch_of_slot = slots % NCHUNK
    act_mask = np.array(
        [_is_act(b, c) for b, c in zip(bt_of_slot, ch_of_slot)], dtype=bool
    )

    sum_a = np.zeros((P, NCHUNK * NBT))
    sum_d = np.zeros((P, NCHUNK * NBT))
    for r in res.results:
        sum_a += r["parts_a"].reshape(P, NCHUNK * NBT).astype(np.float64)
        sum_d += r["parts_d"].reshape(P, NCHUNK * NBT).astype(np.float64)

    pad_d = np.where((ch_of_slot == NCHUNK - 1), NPAD * N_CORES, 0)
    sum_d = sum_d - pad_d[None, :]
    sum_d = np.where(act_mask[None, :], 0.0, sum_d) / ratio
    sum_a = np.where(act_mask[None, :], sum_a, 0.0)

    per_bt = (sum_a + sum_d).reshape(P, NBT, NCHUNK).sum(2)    # [P, NBT]
    rowsum = per_bt.T.reshape(B)                               # row i = bt*128+p

    excl = rowsum - np.exp(scorr)
    L = num - np.log(np.exp(num) + excl)
    return np.float32(-np.mean(L))


# revision 21
# speedup vs baseline: 1.7783x; 1.0786x over previous
"""CosLoss (ArcFace-style margin loss) Trainium2 kernel, 8-way class-sharded.

Math (reference):
    xn   = x / ||x||_row                       [B, D]
    wf   = xn @ W.T                            [B, C]
    corr = wf[i, labels[i]]                    [B]
    num  = S*(corr - M)
    excl = sum_j exp(S*wf[i,j]) - exp(S*corr)
    L    = num - log(exp(num) + excl);  out = -mean(L)

Device does the O(B*C*D) matmul and the O(B*C) exp row-sums; host does the
O(B*D) glue exactly in fp64 (row norms -> rs = S/||x||, the correct-class
dot, and the final scalar combine).

Per core (classes sharded 8 x 4000, padded to 4096):
  - z = x @ W_shard.T via fp8(e4m3) DoubleRow matmuls: the full D=256
    contraction in one PE pass (2 fp8 weights/cell), ~1.7x bf16 throughput.
    x is scaled by SX, W by SW on host; rs/(SX*SW) undoes it in the exp.
  - Row sums of exp(rs*z) are split across two engines working in parallel:
      ACT: exact Exp activation with fused accumulate (scale=rs).
      DVE: custom 8-stage op computing (1 + y/32)^32 ~= exp(y) with a fused
           accumulator (one 1x pass per tile, no separate reduce). The
           systematic bias of the pow-32 approximation is removed on host
           with a single scale factor calibrated on a 256-column sample.
  - Outputs are per-(batch-tile, half) partial sums; host combines.
"""

import math
from contextlib import ExitStack
from operator import add as _op_add

import ml_dtypes
import numpy as np

import concourse.bass as bass
import concourse.mybir as mybir
import concourse.tile as tile
from concourse import bacc
from concourse.bass_utils import run_bass_kernel_spmd

S = 30.0
MARGIN = 0.4
N_CORES = 8
B, D, C = 8192, 256, 32000
CSH = C // N_CORES          # 4000 real classes per core
CPAD = 4096                 # padded shard width
NPAD = CPAD - CSH           # 96 zero-padded classes (always in half 1)
P = 128
NBT = B // P                # 64 batch tiles
HALF = CPAD // 2            # 2048 classes per PSUM tile
SX, SW = 4.0, 32.0          # fp8 pre-scales for x and W
KEXP = 32.0                 # (1 + y/32)^32 fastexp on DVE

FP32 = mybir.dt.float32
BF16 = mybir.dt.bfloat16
FP8 = mybir.dt.float8e4
NP_FP8 = ml_dtypes.float8_e4m3

NCHUNK = 4                  # PSUM chunks per bt
CW = 1024                   # PSUM tile width
# Chunk column ranges: the last chunk stops at CSH=4000 so the zero-padded
# classes are never computed at all.
CHUNKS = [(0, 1024), (1024, 2048), (2048, 3072), (3072, CSH)]


# Consumer of each (bt, chunk): ACT takes {0, 3} (3 is the narrow 928-col
# chunk, balancing ACT's higher per-instruction overhead), DVE takes {1, 2}.
# Measured per-bt busy: ACT ~2.45us, DVE ~2.42us.
def _is_act(bt, ch):
    return ch in (0, 3)


# ---------------------------------------------------------------------------
# Custom DVE op: out = (in0*s0 + s1)^32, accum_out = sum(out) along free dim.
# Body depth 7 (mul, add, 5x square) + accumulator stage 8.
# Registered into concourse.dve_ops at import time (name-keyed registry).
# ---------------------------------------------------------------------------
_POW32_NAME = "POW32_EXP_REDUCE_ANT"


def _pow32_ref(in0, in1, c0, c1, c2):
    b = (in0.astype(np.float32) * c0 + c1).astype(np.float32)
    for _ in range(5):
        b = (b * b).astype(np.float32)
    return b, b.reshape(b.shape[0], -1).sum(axis=-1, keepdims=True)


def _register_pow32_op():
    import concourse.dve_ops as dve_ops
    from concourse.dve_spec import C0, C1, Spec, Src0, Zero, _has_src1, lower, sq
    from concourse.dve_uop import DveOpSpec

    if any(op.name == _POW32_NAME for op in dve_ops.OPS):
        return next(op for op in dve_ops.OPS if op.name == _POW32_NAME)

    body = Src0 * C0 + C1
    for _ in range(5):
        body = sq(body)
    spec = Spec(body=body, accum=_op_add, accum_init=Zero, reference=_pow32_ref)

    row = dve_ops._CUSTOM_DVE_ROW_BASE + len(dve_ops.OPS)
    assert row < 0x20
    shas = {}
    for ver in ("v3", "v4"):
        try:
            s = DveOpSpec(
                name=_POW32_NAME, opcode=row, uops=lower(spec, ver=ver),
                rd1_en=_has_src1(spec),
            )
            shas[ver] = s.sha(ver)
        except Exception:
            pass
    assert "v3" in shas, "pow32 spec failed to lower for TRN2 (v3)"
    op = dve_ops.DveOp(_POW32_NAME, spec, subdim=False, uops_sha=shas)
    dve_ops.OPS.append(op)
    dve_ops.CUSTOM_DVE_SPECS[_POW32_NAME] = spec
    dve_ops._SUB_OPCODE_FOR_NAME[_POW32_NAME] = row
    return op


_POW32_OP = _register_pow32_op()


def _emit(tc, ins, outs):
    nc = tc.nc
    xT8, wT8, rs_sc, rs_k = ins["xT8"], ins["wT8"], ins["rs_sc"], ins["rs_k"]
    parts_a, parts_d = outs["parts_a"], outs["parts_d"]

    with ExitStack() as ctx:
        singles = ctx.enter_context(tc.tile_pool(name="singles", bufs=1))
        scr = ctx.enter_context(tc.tile_pool(name="scr", bufs=2))
        psum = ctx.enter_context(tc.tile_pool(name="psum", bufs=4, space="PSUM"))

        rs_sc_sb = singles.tile([P, NBT], FP32)
        rs_k_sb = singles.tile([P, NBT], FP32)
        wT_sb = singles.tile([P, 2, CPAD], FP8)
        xT_sb = singles.tile([P, 2, B], FP8)
        pa_sb = singles.tile([P, NCHUNK * NBT], FP32)
        pd_sb = singles.tile([P, NCHUNK * NBT], FP32)
        nc.gpsimd.memset(pa_sb, 0.0)
        nc.gpsimd.memset(pd_sb, 0.0)

        # Prologue DMAs spread across engine DGE queues (each queue moves
        # only ~26 GB/s); ordered so bt 0's deps land first. All host<->dev
        # layouts are SBUF-native [P, ...] so every DMA is contiguous per
        # partition (element-scatter patterns cost ~60us in descriptors).
        # First the deps of bt 0 chunk 0: wT cols 0-1023 split across two
        # queues, and xT rows 0-1023 on a third.
        nc.sync.dma_start(out=wT_sb[:, :, 0:512], in_=wT8[:, :, 0:512])
        nc.scalar.dma_start(out=wT_sb[:, :, 512:1024], in_=wT8[:, :, 512:1024])
        nc.gpsimd.dma_start(out=xT_sb[:, :, 0:1024], in_=xT8[:, :, 0:1024])
        nc.sync.dma_start(out=rs_sc_sb, in_=rs_sc)
        nc.sync.dma_start(out=rs_k_sb, in_=rs_k)
        wt_engines = [nc.scalar, nc.sync, nc.scalar]
        for j, eng in enumerate(wt_engines):
            lo = 1024 + j * CW
            eng.dma_start(
                out=wT_sb[:, :, lo : lo + CW], in_=wT8[:, :, lo : lo + CW]
            )
        for c in range(1, 8):
            lo, hi = c * (B // 8), (c + 1) * (B // 8)
            eng = nc.gpsimd if c % 2 == 0 else nc.sync
            eng.dma_start(out=xT_sb[:, :, lo:hi], in_=xT8[:, :, lo:hi])

        for bt in range(NBT):
            lhs = xT_sb[:, :, bt * P : (bt + 1) * P]
            for ch in range(NCHUNK):
                clo, chi = CHUNKS[ch]
                w = chi - clo
                pt = psum.tile([P, CW], FP32, tag="pt")
                for j0 in range(clo, chi, 512):
                    j1 = min(j0 + 512, chi)
                    nc.tensor.matmul(
                        pt[:, j0 - clo : j1 - clo],
                        lhsT=lhs,
                        rhs=wT_sb[:, :, j0:j1],
                        start=True,
                        stop=True,
                        perf_mode=mybir.MatmulPerfMode.DoubleRow,
                    )
                slot = NCHUNK * bt + ch
                if _is_act(bt, ch):
                    et = scr.tile([P, CW], BF16, tag="et")
                    nc.scalar.activation(
                        out=et[:, :w], in_=pt[:, :w],
                        func=mybir.ActivationFunctionType.Exp,
                        scale=rs_sc_sb[:, bt : bt + 1],
                        accum_out=pa_sb[:, slot : slot + 1],
                    )
                else:
                    ft = scr.tile([P, CW], BF16, tag="ft")
                    nc.vector._custom_dve(
                        _POW32_OP,
                        out=ft[:, :w],
                        in0=pt[:, :w],
                        s0=rs_k_sb[:, bt : bt + 1],
                        s1=1.0,
                        accum_out=pd_sb[:, slot : slot + 1],
                    )

        nc.sync.dma_start(out=parts_a, in_=pa_sb)
        nc.scalar.dma_start(out=parts_d, in_=pd_sb)


def _build():
    nc = bacc.Bacc("TRN2", target_bir_lowering=False, debug=False)
    ins = {
        "xT8": nc.dram_tensor("xT8", [P, 2, B], FP8, kind="ExternalInput").ap(),
        "wT8": nc.dram_tensor("wT8", [P, 2, CPAD], FP8, kind="ExternalInput").ap(),
        "rs_sc": nc.dram_tensor("rs_sc", [P, NBT], FP32, kind="ExternalInput").ap(),
        "rs_k": nc.dram_tensor("rs_k", [P, NBT], FP32, kind="ExternalInput").ap(),
    }
    outs = {
        "parts_a": nc.dram_tensor(
            "parts_a", [P, NCHUNK * NBT], FP32, kind="ExternalOutput"
        ).ap(),
        "parts_d": nc.dram_tensor(
            "parts_d", [P, NCHUNK * NBT], FP32, kind="ExternalOutput"
        ).ap(),
    }
    with tile.TileContext(nc) as tc:
        _emit(tc, ins, outs)
    nc.compile()
    return nc


_NC_CACHE = {}


def _get_nc():
    if "nc" not in _NC_CACHE:
        _NC_CACHE["nc"] = _build()
    return _NC_CACHE["nc"]


def _install_trace_hook():
    """Make `antenv.axon_hooks` importable so run_bass_kernel_spmd(trace=True)
    can capture NTFF profiles under axon. Returns False if unavailable."""
    try:
        from antenv.axon_hooks import get_axon_ntff_profile_hook  # noqa: F401

        return True
    except ImportError:
        pass
    try:
        import sys
        import types

        from trn_agent_boot.trn_boot import _ntff_profile_via_ctypes

        hook = _ntff_profile_via_ctypes("/opt/axon/libaxon_pjrt.so")
        if hook is None:
            return False
        mod = types.ModuleType("antenv.axon_hooks")
        mod._hook = hook
        mod.get_axon_ntff_profile_hook = lambda: mod._hook
        mod.set_axon_ntff_profile_hook = lambda h: setattr(mod, "_hook", h)
        sys.modules["antenv.axon_hooks"] = mod
        import antenv

        antenv.axon_hooks = mod
        return True
    except Exception:
        return False


def _pack_T(a8):
    """[N, 256] fp8 row-major -> [128, 2, N] DoubleRow operand layout:
    out[p, kk, n] = a8[n, kk*128 + p]."""
    n = a8.shape[0]
    return np.ascontiguousarray(a8.reshape(n, 2, P).transpose(2, 1, 0))


def _fastexp_pow32(y):
    b = (y.astype(np.float32) * np.float32(1.0 / KEXP) + np.float32(1.0)).astype(
        np.float32
    )
    for _ in range(5):
        b = (b * b).astype(np.float32)
    return b


def kernel(x, labels, W, trace=False):
    x = np.ascontiguousarray(np.asarray(x, dtype=np.float32))
    W = np.ascontiguousarray(np.asarray(W, dtype=np.float32))
    labels_i = np.asarray(labels).astype(np.int64)

    # Host glue (exact, O(B*D)): row norms, rs, correct-class logit.
    ssq = np.einsum("bd,bd->b", x.astype(np.float64), x.astype(np.float64))
    rs = S / np.sqrt(ssq)                                     # [B] fp64
    dotg = np.einsum(
        "bd,bd->b", x.astype(np.float64), W[labels_i].astype(np.float64)
    )
    scorr = rs * dotg
    num = scorr - S * MARGIN

    # Device operands.
    x8 = (x * SX).astype(NP_FP8)
    xT8 = _pack_T(x8)
    rs_dev = (rs / (SX * SW)).astype(np.float32)
    # [P, NBT] SBUF-native layout: [p, bt] = rs[bt*128 + p]
    rs_sc = np.ascontiguousarray(rs_dev.reshape(NBT, P).T)
    rs_k = np.ascontiguousarray((rs_dev / KEXP).astype(np.float32).reshape(NBT, P).T)

    in_maps = []
    w8_f32 = None
    for k in range(N_CORES):
        w8k = (W[k * CSH : (k + 1) * CSH] * SW).astype(NP_FP8)
        if k == 0:
            w8_f32 = w8k.astype(np.float32)  # for calibration sampling
        w8p = np.zeros((CPAD, D), dtype=NP_FP8)
        w8p[:CSH] = w8k
        in_maps.append(
            {"xT8": xT8, "wT8": _pack_T(w8p), "rs_sc": rs_sc, "rs_k": rs_k}
        )

    nc = _get_nc()
    if trace and not _install_trace_hook():
        trace = False
    res = run_bass_kernel_spmd(nc, in_maps, core_ids=list(range(N_CORES)), trace=trace)
    if trace and res.exec_time_ns is not None:
        print(f"HW exec time: {res.exec_time_ns} ns")

    # Calibrate the pow-32 fastexp bias on a 256-column sample of core 0's
    # shard (same fp8 values the device saw).
    x8_f32 = x8.astype(np.float32)
    cols = np.arange(0, CSH, CSH // 256)[:256]
    z_s = x8_f32 @ w8_f32[cols].T                              # [B, 256] fp32
    y_s = rs_dev[:, None] * z_s
    ratio = float(_fastexp_pow32(y_s).astype(np.float64).sum()) / float(
        np.exp(y_s.astype(np.float64)).sum()
    )

    # Combine per-(bt, chunk) partials. Chunk ranges stop at CSH so no
    # padded classes were ever computed.
    slots = np.arange(NCHUNK * NBT)
    bt_of_slot = slots // NCHUNK
    ch_of_slot = slots % NCHUNK
    act_mask = np.array(
        [_is_act(b, c) for b, c in zip(bt_of_slot, ch_of_slot)], dtype=bool
    )

    sum_a = np.zeros((P, NCHUNK * NBT))
    sum_d = np.zeros((P, NCHUNK * NBT))
    for r in res.results:
        sum_a += r["parts_a"].reshape(P, NCHUNK * NBT).astype(np.float64)
        sum_d += r["parts_d"].reshape(P, NCHUNK * NBT).astype(np.float64)

    sum_d = np.where(act_mask[None, :], 0.0, sum_d) / ratio
    sum_a = np.where(act_mask[None, :], sum_a, 0.0)

    per_bt = (sum_a + sum_d).reshape(P, NBT, NCHUNK).sum(2)    # [P, NBT]
    rowsum = per_bt.T.reshape(B)                               # row i = bt*128+p

    excl = rowsum - np.exp(scorr)
    L = num - np.log(np.exp(num) + excl)
    return np.float32(-np.mean(L))
